# revision 30
# baseline (speedup 1.0000x reference)
"""Trainium2 Bass kernel for per-gene linear layer.

Math (reference):
    gene    = x[:, :20000]           # (B, G)
    nongene = x[:, 20000:]           # (B, K=128)
    y[:, g] = gene[:, g] * W[g, 0] + nongene @ W[g, 1:] + b[g]

Sharding: model parallel over genes across 8 cores (2500 genes each,
padded to 2560 = 20 tiles of 128 for uniform SPMD tiling).
Per-core device layout keeps genes on the partition axis ([G, B] output):

Per gene tile (128 genes x 1024 batch):
    psum  = wshT.T @ xnT            (TensorE, float32r: full rate, ~tf32 precision)
    t     = psum + b[:, None]       (per-partition bias; ACT or DVE, rotating)
    out   = xg * dw[:, None] + t    (fused fma; DVE or GPSIMD, rotating)

xg is loaded as bf16 (it only feeds the small diagonal term); the dominant
matmul term runs in float32r from f32 data. DMAs are batched into 1 MB
supertiles (4 gene tiles per load, 2 per store) and loads/stores are issued
on the two separate HWDGE rings (SP and ACT) to avoid head-of-line blocking.
"""

import os
import numpy as np
from contextlib import ExitStack

import concourse.bass as bass
import concourse.tile as tile
from concourse import bacc, mybir
from concourse.bass_utils import run_bass_kernel_spmd

B = 1024           # batch
G = 20000          # genes (output dim)
K = 128            # shared nongene features
IN_DIM = G + K     # 20128
N_CORES = 8
G_CORE = G // N_CORES            # 2500 genes per core
N_GT = 20                        # gene tiles per core (padded)
G_PAD = N_GT * 128               # 2560
ST_LOAD = 4                      # gene tiles per load DMA  (1 MB bf16)
ST_STORE = 2                     # gene tiles per store DMA (1 MB f32)

_NC_CACHE = None
LAST_RESULTS = None  # BassKernelResults of the most recent run (for test harness)


def _build_nc():
    nc = bacc.Bacc("TRN2", target_bir_lowering=False, debug=False,
                   enable_asserts=True, num_devices=N_CORES)
    f32 = mybir.dt.float32
    f32r = mybir.dt.float32r  # 4-byte storage, reduced-precision PE mode
    bf16 = mybir.dt.bfloat16

    xgT = nc.dram_tensor("xgT", [G_PAD, B], bf16, kind="ExternalInput").ap()
    wshT = nc.dram_tensor("wshT", [K, G_PAD], f32r, kind="ExternalInput").ap()
    xnT = nc.dram_tensor("xnT", [K, B], f32r, kind="ExternalInput").ap()
    dwt = nc.dram_tensor("dwt", [128, N_GT], f32, kind="ExternalInput").ap()
    bt = nc.dram_tensor("bt", [128, N_GT], f32, kind="ExternalInput").ap()
    yT = nc.dram_tensor("yT", [G_PAD, B], f32, kind="ExternalOutput").ap()

    with tile.TileContext(nc) as tc, ExitStack() as ctx:
        const = ctx.enter_context(tc.tile_pool(name="const", bufs=1))
        xg_pool = ctx.enter_context(tc.tile_pool(name="xg", bufs=3))
        t_pool = ctx.enter_context(tc.tile_pool(name="t", bufs=4))
        out_pool = ctx.enter_context(tc.tile_pool(name="out", bufs=4))
        psum_pool = ctx.enter_context(
            tc.tile_pool(name="psum", bufs=4, space="PSUM"))

        # wshT loaded in per-supertile chunks so tile 0's weights arrive
        # within the first ~2 us instead of after the full 1.3 MB transfer
        n_sup = N_GT // ST_LOAD
        wsh_c = []
        for s in range(n_sup):
            wc = const.tile([K, ST_LOAD * 128], f32r, tag=f"wsh{s}")
            wsh_c.append(wc)
        nc.sync.dma_start(wsh_c[0][:], wshT[:, 0:ST_LOAD * 128])
        xn_s = const.tile([K, B], f32r)
        nc.scalar.dma_start(xn_s[:], xnT[:])
        # remaining weight chunks as one block on the (still idle) ACT ring
        for s in range(1, n_sup):
            nc.scalar.dma_start(
                wsh_c[s][:],
                wshT[:, s * ST_LOAD * 128:(s + 1) * ST_LOAD * 128])
        dw_s = const.tile([128, N_GT], f32)
        nc.gpsimd.dma_start(dw_s[:], dwt[:])
        b_s = const.tile([128, N_GT], f32)
        nc.gpsimd.dma_start(b_s[:], bt[:])

        # warm the ACT function table during the DMA head so the first real
        # ACTIVATE doesn't eat the ~1.3us table load
        warm = const.tile([128, 1], f32)
        nc.gpsimd.memset(warm[:], 0.0)
        warm2 = const.tile([128, 1], f32)
        nc.scalar.activation(warm2[:], warm[:],
                             mybir.ActivationFunctionType.Identity,
                             bias=0.0, scale=1.0)

        for s in range(n_sup):
            xg_sup = xg_pool.tile([128, ST_LOAD, B], bf16)
            src = xgT[s * ST_LOAD * 128:(s + 1) * ST_LOAD * 128, :].rearrange(
                "(j p) e -> p j e", p=128)
            if s < 2:
                # early phase: no stores in flight yet -- split the load
                # across both HWDGE rings to use the idle one
                nc.sync.dma_start(xg_sup[:, 0:2, :], src[:, 0:2, :])
                nc.scalar.dma_start(xg_sup[:, 2:4, :], src[:, 2:4, :])
            else:
                nc.sync.dma_start(xg_sup[:], src)

            for jj in range(ST_LOAD // ST_STORE):
                out_sup = out_pool.tile([128, ST_STORE, B], f32)
                for j2 in range(ST_STORE):
                    lt = jj * ST_STORE + j2      # tile index in load supertile
                    gt = s * ST_LOAD + lt        # global gene tile index
                    g0 = gt * 128

                    psum = psum_pool.tile([128, B], f32)
                    wl = wsh_c[s][:, lt * 128:(lt + 1) * 128]
                    for h in range(2):
                        c0 = h * 512
                        nc.tensor.matmul(psum[:, c0:c0 + 512],
                                         wl,
                                         xn_s[:, c0:c0 + 512],
                                         start=True, stop=True)

                    # t = psum + b  (ScalarE PSUM->SBUF, per-partition bias)
                    t = t_pool.tile([128, B], f32)
                    nc.scalar.activation(t[:], psum[:],
                                         mybir.ActivationFunctionType.Identity,
                                         bias=b_s[:, gt:gt + 1], scale=1.0)

                    # out = (xg * dw) + t  -- one fused DVE pass, all-SBUF
                    nc.vector.scalar_tensor_tensor(
                        out_sup[:, j2, :], xg_sup[:, lt, :],
                        dw_s[:, gt:gt + 1], t[:],
                        op0=mybir.AluOpType.mult, op1=mybir.AluOpType.add)

                dst = yT[(s * ST_LOAD + jj * ST_STORE) * 128:
                         (s * ST_LOAD + (jj + 1) * ST_STORE) * 128, :].rearrange(
                    "(j p) e -> p j e", p=128)
                si = s * (ST_LOAD // ST_STORE) + jj
                if si >= 8:
                    # tail phase: loads all issued -- split the store
                    # across both HWDGE rings
                    nc.scalar.dma_start(dst[:, 0:1, :], out_sup[:, 0:1, :])
                    nc.sync.dma_start(dst[:, 1:2, :], out_sup[:, 1:2, :])
                else:
                    # stores on the ACT HWDGE ring; loads own the SP ring
                    nc.scalar.dma_start(dst, out_sup[:])

    nc.compile()
    return nc


def _get_nc():
    global _NC_CACHE
    if _NC_CACHE is None:
        _NC_CACHE = _build_nc()
    return _NC_CACHE


def kernel(x, W, b):
    global LAST_RESULTS
    import ml_dtypes
    x = np.asarray(x, dtype=np.float32)
    W = np.asarray(W, dtype=np.float32)
    b = np.asarray(b, dtype=np.float32)
    assert x.shape == (B, IN_DIM) and W.shape == (G, 1 + K) and b.shape == (G,)

    xT = np.ascontiguousarray(x.T)          # (20128, 1024)
    xnT = np.ascontiguousarray(xT[G:])      # (128, 1024), replicated
    # gene block as bf16 (feeds only the small diagonal term), padded per core
    xg_pad = np.zeros((N_CORES, G_PAD, B), ml_dtypes.bfloat16)
    xg_pad[:, :G_CORE] = xT[:G].astype(ml_dtypes.bfloat16).reshape(
        N_CORES, G_CORE, B)

    in_maps = []
    for c in range(N_CORES):
        g0 = c * G_CORE
        Wc = W[g0:g0 + G_CORE]

        def cols(v):
            m = np.zeros((128, N_GT), np.float32)
            m[:, :G_CORE // 128] = v[:(G_CORE // 128) * 128].reshape(-1, 128).T
            rem = G_CORE - (G_CORE // 128) * 128
            if rem:
                m[:rem, G_CORE // 128] = v[(G_CORE // 128) * 128:]
            return m

        wsh = np.zeros((K, G_PAD), np.float32)
        wsh[:, :G_CORE] = Wc[:, 1:].T
        in_maps.append({
            "xgT": xg_pad[c],
            "wshT": wsh,
            "xnT": xnT,
            "dwt": cols(np.ascontiguousarray(Wc[:, 0])),
            "bt": cols(np.ascontiguousarray(b[g0:g0 + G_CORE])),
        })

    nc = _get_nc()
    trace = bool(os.environ.get("KERNEL_TRACE"))
    kwargs = {}
    if trace:
        tdir = os.environ.get("KERNEL_TRACE_DIR")
        if tdir:
            os.makedirs(tdir, exist_ok=True)
            kwargs["tmpdir"] = tdir
    LAST_RESULTS = run_bass_kernel_spmd(nc, in_maps, list(range(N_CORES)),
                                        trace=trace, **kwargs)
    y = np.empty((B, G), np.float32)
    yT_view = y.T  # fill transposed view to avoid a second big copy
    for c in range(N_CORES):
        yT_view[c * G_CORE:(c + 1) * G_CORE] = \
            LAST_RESULTS.results[c]["yT"][:G_CORE]
    return y
